# revision 51
# baseline (speedup 1.0000x reference)
"""Trainium2 Bass kernel for nn_CascadeTransformerMM (4-layer ternary-GLU cascade).

Math (per layer, per token row):
  h   = rms_scale * x * rsqrt(mean(x^2) + 1e-6)
  s   = clip(127/(max|h| + 1e-5), 1e-3, 1e3);  q = round(s*h)      (ints in [-127,127])
  Wt  = clip(round(W * 127/(max|W| + 1e-5)), -1, 1)                 (ternary {-1,0,1})
  u   = (q @ Wg_t)/s ; v = (q @ Wu_t)/s ; g = silu(u)*v
  s2  = clip(127/(max|g| + 1e-5), 1e-3, 1e3); gq = round(s2*g)
  x  += (gq @ Wd_t)/s2

Distribution: pure data-parallel over the batch dim (8 batches -> 8 cores),
weights replicated per core. Per-matrix |W|max is computed cooperatively:
each core reduces a 1/8 row-slice, then two tiny AllReduce(max) ops (layer 0
first so ternarization can start early; layers 1-3 overlapped with layer-0
compute) share the 12 scalars.

Schedule: x stays SBUF-resident for all 4 layers. One software-pipelined
stream of 32 tile-steps; step t emits up-proj(t) then down-proj(t-1) on the
PE so the g-quant + transpose latency of tile t hides under other matmuls.
Stats+quant for tile t+1 run in the up-proj(t) shadow. Weight ternarization
(fp32 -> fp8 {-1,0,1}) runs on the otherwise-idle GPSIMD engine, spread
across the steps of the previous layer, staged via DRAM; SBUF weight tiles
are chunked so boundary refills overlap the previous layer's tail."""

import os
import sys

for _p in ("/opt/trn_rl_repo", "/root/.axon_site/_ro/trn_rl_repo"):
    if os.path.isdir(_p) and _p not in sys.path:
        sys.path.insert(0, _p)

import numpy as np
from contextlib import ExitStack

import concourse.bass as bass
import concourse.mybir as mybir
import concourse.tile as tile
from concourse.bass_utils import run_bass_kernel_spmd

dt = mybir.dt
AF = mybir.ActivationFunctionType
ALU = mybir.AluOpType

MAGIC = float(1.5 * 2**23)  # fp32 round-to-nearest-even magic constant
D = 1024
F = 4096
L = 4
NCORES = 8
TOK = 1024  # tokens per core (one batch of S=1024)

NDK = D // 128   # 8 contraction tiles for up-proj
NFT = F // 128   # 32 contraction tiles for down-proj
NFC = F // 512   # 8 free-dim chunks for up-proj
NTT = TOK // 128  # 8 token tiles
NCH = F // 1024  # 4 weight chunks (wg/wu) / quarters (wd)


def _split_excess_waits(nc, max_waits: int = 1) -> int:
    """walrus in this container rejects >1 sync-wait per instruction; split
    extras into standalone event-semaphore waits on the same engine (same-
    engine program order makes this semantically identical)."""
    n = 0
    for func in nc.m.functions:
        for block in func.blocks:
            changed = False
            out = []
            for inst in block.instructions:
                si = getattr(inst, "sync_info", None)
                if si is not None and si.on_wait and len(si.on_wait) > max_waits:
                    waits = list(si.on_wait)
                    for j, w in enumerate(waits[max_waits:]):
                        out.append(
                            mybir.InstEventSemaphore(
                                name=f"{inst.name}-xw{j}",
                                engine=inst.engine,
                                ins=[],
                                outs=[],
                                sync_info=mybir.SyncInfo(on_wait=[w], on_update=[]),
                            )
                        )
                        n += 1
                    inst.sync_info = mybir.SyncInfo(
                        on_wait=waits[:max_waits], on_update=list(si.on_update)
                    )
                    changed = True
                out.append(inst)
            if changed:
                block.instructions = out
    return n


def build(n_cores: int = NCORES, n_tok_tiles: int = NTT, n_layers: int = L,
          no_collectives: bool = False, debug_dump: bool = False) -> bass.Bass:
    nc = bass.Bass(num_devices=n_cores)
    _dbg_tensors = {}

    def _dbg(name, ap, dtype, shape):
        if not debug_dump:
            return
        d = nc.dram_tensor(name, shape, dtype)
        _dbg_tensors[name] = d
        nc.sync.dma_start(d[:], ap)
    tok = n_tok_tiles * 128

    x_ext = nc.declare_dram_parameter("x", [tok, D], dt.float32, isOutput=False)
    rs_ext = nc.declare_dram_parameter("rs", [n_layers, D], dt.float32, isOutput=False)
    # wg/wu repacked host-side to [L, F//1024, NDK, 128, 1024] so every
    # [128, 1024] weight tile is one contiguous 512 KB DMA; wd's row-slabs
    # are naturally contiguous.
    wg_ext = nc.declare_dram_parameter("wg", [n_layers, NCH, NDK, 128, 1024], dt.float32, isOutput=False)
    wu_ext = nc.declare_dram_parameter("wu", [n_layers, NCH, NDK, 128, 1024], dt.float32, isOutput=False)
    wd_ext = nc.declare_dram_parameter("wd", [n_layers, F, D], dt.float32, isOutput=False)
    # per-core row-slices of each matrix for the cooperative |W|max,
    # as [L, nun, 128, 1024] contiguous units
    nun = (D // n_cores) * F // (128 * 1024)
    slg_ext = nc.declare_dram_parameter("slg", [n_layers, nun, 128, 1024], dt.float32, isOutput=False)
    slu_ext = nc.declare_dram_parameter("slu", [n_layers, nun, 128, 1024], dt.float32, isOutput=False)
    sld_ext = nc.declare_dram_parameter("sld", [n_layers, nun, 128, 1024], dt.float32, isOutput=False)
    out_ext = nc.declare_dram_parameter("out", [tok, D], dt.float32, isOutput=True)

    mx_loc1 = nc.dram_tensor("mx_loc1", [1, 16], dt.float32)
    mx_glob1 = nc.dram_tensor("mx_glob1", [1, 16], dt.float32)
    mx_loc2 = nc.dram_tensor("mx_loc2", [1, 16], dt.float32)
    mx_glob2 = nc.dram_tensor("mx_glob2", [1, 16], dt.float32)

    nsteps = n_layers * n_tok_tiles

    with tile.TileContext(nc) as tc, ExitStack() as ctx:
        P = ctx.enter_context
        wts = P(tc.tile_pool(name="wts", bufs=1))       # fp8 weight chunk tiles
        xp = P(tc.tile_pool(name="xres", bufs=1))
        gp = P(tc.tile_pool(name="g", bufs=1))
        gqp = P(tc.tile_pool(name="gq", bufs=1))
        gqtp = P(tc.tile_pool(name="gqt", bufs=2))
        t1p = P(tc.tile_pool(name="t1", bufs=1))
        qp = P(tc.tile_pool(name="q", bufs=1))
        qtp = P(tc.tile_pool(name="qt", bufs=2))
        silup = P(tc.tile_pool(name="silu", bufs=1))
        wsp = P(tc.tile_pool(name="wstream", bufs=2))
        wmp = P(tc.tile_pool(name="wmstream", bufs=2))
        wi8p = P(tc.tile_pool(name="wi8", bufs=2))
        w8sp = P(tc.tile_pool(name="w8s", bufs=2))
        scbp = P(tc.tile_pool(name="scb", bufs=1))
        scp = P(tc.tile_pool(name="sc", bufs=4))
        constp = P(tc.tile_pool(name="const", bufs=1))
        dram = P(tc.tile_pool(name="dram", bufs=2, space="DRAM"))
        psA = P(tc.tile_pool(name="psA", bufs=2, space="PSUM"))
        psB = P(tc.tile_pool(name="psB", bufs=2, space="PSUM"))
        psD = P(tc.tile_pool(name="psD", bufs=4, space="PSUM"))

        # ---------- constants ----------
        ones1 = constp.tile([1, 128], dt.float32, tag="ones1")
        nc.gpsimd.memset(ones1[:], 1.0)
        mag = constp.tile([128, 1], dt.float32, tag="mag")
        nc.gpsimd.memset(mag[:], MAGIC)
        nmag = constp.tile([128, 1], dt.float32, tag="nmag")
        nc.gpsimd.memset(nmag[:], -MAGIC)

        def small(tag):
            return scp.tile([128, 1], dt.float32, tag=tag, name=tag)

        # ---------- cooperative per-matrix |W|max (two phases) ----------
        wmax_cols = constp.tile([128, 16], dt.float32, tag="wmaxc")
        nc.gpsimd.memset(wmax_cols[:], 0.0)

        _wm_alt = [0]

        def emit_wmax_unit(l, mi, ext):
            idx = 3 * l + mi
            part = scp.tile([128, nun], dt.float32, tag="wmaxpart")
            for un in range(nun):
                wt = wmp.tile([128, 1024], dt.float32, tag="wmstream")
                ring = nc.gpsimd if _wm_alt[0] % 2 == 0 else nc.scalar
                _wm_alt[0] += 1
                ring.dma_start(wt[:], ext[l, un])
                nc.vector.tensor_reduce(
                    part[:, un:un + 1], wt[:], axis=mybir.AxisListType.X,
                    op=ALU.max, apply_absolute_value=True,
                )
            nc.vector.tensor_reduce(
                wmax_cols[:, idx:idx + 1], part[:, 0:nun],
                axis=mybir.AxisListType.X, op=ALU.max,
                apply_absolute_value=False,
            )

        def emit_wmax_allreduce(tag, mloc, mglob, ncols):
            mrow = constp.tile([1, 16], dt.float32, tag=f"mrow{tag}")
            nc.gpsimd.memset(mrow[:], 0.0)
            nc.gpsimd.tensor_reduce(
                mrow[:, 0:ncols], wmax_cols[:, 0:ncols], axis=mybir.AxisListType.C,
                op=ALU.max,
            )
            nc.sync.dma_start(mloc[:], mrow[:])
            if no_collectives:
                # sim-debug mode: local max stands in for the global one
                nc.sync.dma_start(mglob[:], mloc[:])
            else:
                nc.gpsimd.collective_compute(
                    "AllReduce",
                    ALU.max,
                    replica_groups=[list(range(n_cores))],
                    ins=[mloc[:].opt()],
                    outs=[mglob[:].opt()],
                )

        def emit_wmax_bcast(tag, mglob):
            grow = constp.tile([1, 16], dt.float32, tag=f"grow{tag}")
            nc.sync.dma_start(grow[:], mglob[:])
            bc = psD.tile([128, 512], dt.float32, tag="xdps")
            nc.tensor.matmul(bc[:, 0:16], ones1[:], grow[:], start=True, stop=True)
            msc = constp.tile([128, 16], dt.float32, tag=f"msc{tag}")
            nc.vector.tensor_scalar(msc[:], bc[:, 0:16], 1e-5, None, op0=ALU.add)
            mrec = constp.tile([128, 16], dt.float32, tag=f"mrec{tag}")
            nc.vector.reciprocal(mrec[:], msc[:])
            wsc = constp.tile([128, 16], dt.float32, tag=f"wsc{tag}")
            nc.vector.tensor_scalar(wsc[:], mrec[:], 127.0, None, op0=ALU.mult)
            return wsc

        # phase 1: layer 0 only (3 scalars) -> ternarize layer 0 can start early
        for mi, ext in enumerate((slg_ext, slu_ext, sld_ext)):
            emit_wmax_unit(0, mi, ext)
        emit_wmax_allreduce("1", mx_loc1, mx_glob1, 3)
        wsc1 = emit_wmax_bcast("1", mx_glob1)
        # phase 2 (layers 1-3) is spread over steps 0-2 to keep the gpsimd
        # queue free for layer-0 ternarize; AllReduce #2 lands in step 2.
        wsc2_box = [None]

        def wsc_ap(idx):
            return (wsc1 if idx < 3 else wsc2_box[0])[:, idx:idx + 1]

        # ---------- x resident load ----------
        xres = xp.tile([128, n_tok_tiles, D], dt.float32, tag="xres")
        for i in range(n_tok_tiles):
            nc.sync.dma_start(xres[:, i, :], x_ext[i * 128:(i + 1) * 128, :])

        # ---------- scale_bc for layer 0 ----------
        SCB = {}

        def emit_scale_bc(l):
            rrow = t1p.tile([128, D], dt.float32, tag="t1")
            nc.sync.dma_start(rrow[0:1, :], rs_ext[l:l + 1, :])
            scb = scbp.tile([128, D], dt.float32, tag="scb")
            for h in range(2):
                bc = psD.tile([128, 512], dt.float32, tag="xdps")
                nc.tensor.matmul(
                    bc[:], ones1[:], rrow[0:1, h * 512:(h + 1) * 512],
                    start=True, stop=True,
                )
                nc.scalar.activation(scb[:, h * 512:(h + 1) * 512], bc[:], AF.Identity)
            SCB[l] = scb

        emit_scale_bc(0)

        # ---------- weight tiles / ternarize ----------
        # W[l] = {"wg": [4 tiles of [128, NDK, 1024]], "wu": ..., "wd": [4 tiles of [128, 8, 1024]]}
        W = {}
        STG = {}  # DRAM fp8 staging for layers >= 1

        def tern_unit(src_ap, dst_ap, idx, mode):
            # p1 fp32 -> int8 convert (RNE, values <= 127 so exact); p2 is
            # clip(int,-1,1) == sign(int). Engine per `mode`: 'dve' both on
            # DVE (min/max), 'act' p1 DVE + p2 Act AF.Sign, 'gpsimd' both on
            # the (slow but otherwise idle) gpsimd.
            wt = wsp.tile([128, 1024], dt.float32, tag="wstream")
            nc.sync.dma_start(wt[:], src_ap)
            w8i = wi8p.tile([128, 1024], dt.int8, tag="wi8")
            if mode == "gpsimd":
                nc.gpsimd.tensor_scalar(w8i[:], wt[:], wsc_ap(idx), None, op0=ALU.mult)
                nc.gpsimd.tensor_scalar(dst_ap, w8i[:], 1, -1, op0=ALU.min, op1=ALU.max)
            else:
                nc.vector.tensor_scalar(w8i[:], wt[:], wsc_ap(idx), None, op0=ALU.mult)
                if mode == "act":
                    nc.scalar.activation(dst_ap, w8i[:], AF.Sign)
                else:
                    nc.vector.tensor_scalar(dst_ap, w8i[:], 1, -1, op0=ALU.min, op1=ALU.max)

        def tern_unit_staged(src_ap, stage_ap, idx, mode):
            stg = w8sp.tile([128, 1024], dt.float8e4, tag="w8s")
            tern_unit(src_ap, stg[:], idx, mode)
            nc.gpsimd.dma_start(stage_ap, stg[:])

        # layer 0: direct-to-SBUF. wg/wu on gpsimd (loads on sync ring).
        W[0] = {
            "wg": [wts.tile([128, NDK, 1024], dt.float8e4, tag=f"wg{ch}", name=f"wg{ch}") for ch in range(NCH)],
            "wu": [wts.tile([128, NDK, 1024], dt.float8e4, tag=f"wu{ch}", name=f"wu{ch}") for ch in range(NCH)],
            "wd": [wts.tile([128, 8, 1024], dt.float8e4, tag=f"wd{q}", name=f"wd{q}") for q in range(NCH)],
        }
        # layer-0 wd deferred into steps 0-1 (DVE+Act, loads on scalar ring)
        l0_wd_units = [
            (wd_ext[0, ft * 128:(ft + 1) * 128, :], W[0]["wd"][ft // 8][:, ft % 8, :])
            for ft in range(NFT)
        ]

        # tern work lists for layers >= 1 (units spread over prev layer's steps)
        def make_stage_tiles(l):
            STG[l] = {
                "wg": [dram.tile([128, NDK, 1024], dt.float8e4, tag=f"swg{ch}", name=f"swg{ch}") for ch in range(NCH)],
                "wu": [dram.tile([128, NDK, 1024], dt.float8e4, tag=f"swu{ch}", name=f"swu{ch}") for ch in range(NCH)],
                "wd": [dram.tile([128, 8, 1024], dt.float8e4, tag=f"swd{q}", name=f"swd{q}") for q in range(NCH)],
            }

        def tern_units_for_layer(l):
            units = []
            for ch in range(NCH):
                for ext, key, mi in ((wg_ext, "wg", 0), (wu_ext, "wu", 1)):
                    for dk in range(NDK):
                        units.append((ext[l, ch, dk], STG[l][key][ch][:, dk, :], 3 * l + mi))
            for ft in range(NFT):
                units.append((
                    wd_ext[l, ft * 128:(ft + 1) * 128, :],
                    STG[l]["wd"][ft // 8][:, ft % 8, :],
                    3 * l + 2,
                ))
            return units

        # ---------- per-tile stats + activation quant chain ----------
        CH = {}   # t -> (qT, rs)
        DN = {}   # t -> (gqT, rs2)

        def emit_chain(t):
            l, i = divmod(t, n_tok_tiles)
            xsl = xres[:, i, :]
            scb = SCB[l]
            t1 = t1p.tile([128, D], dt.float32, tag="t1")
            ssq = small("ssq")
            nc.scalar.activation(t1[:], xsl, AF.Square, accum_out=ssq[:])
            nc.vector.tensor_tensor(t1[:], xsl, scb[:], op=ALU.mult)
            mx = small("mx")
            nc.vector.tensor_reduce(
                mx[:], t1[:], axis=mybir.AxisListType.X, op=ALU.max,
                apply_absolute_value=True,
            )
            ms = small("ms")
            nc.vector.tensor_scalar(ms[:], ssq[:], 1.0 / D, 1e-6, op0=ALU.mult, op1=ALU.add)
            rt = small("rt")
            nc.scalar.activation(rt[:], ms[:], AF.Sqrt)
            rstd = small("rstd")
            nc.vector.reciprocal(rstd[:], rt[:])
            # one Newton step: rstd *= 1.5 - 0.5*ms*rstd^2  (fixes the ~7e-6
            # Sqrt-LUT error that quantization tie-flips amplify layer by layer)
            nwt = small("nwt")
            nc.vector.tensor_tensor(nwt[:], rstd[:], rstd[:], op=ALU.mult)
            nc.vector.tensor_tensor(nwt[:], nwt[:], ms[:], op=ALU.mult)
            nc.vector.tensor_scalar(nwt[:], nwt[:], -0.5, 1.5, op0=ALU.mult, op1=ALU.add)
            nc.vector.tensor_tensor(rstd[:], rstd[:], nwt[:], op=ALU.mult)
            maxh = small("maxh")
            nc.vector.tensor_tensor(maxh[:], mx[:], rstd[:], op=ALU.mult)
            nc.vector.tensor_scalar(maxh[:], maxh[:], 1e-5, None, op0=ALU.add)
            sr = small("sr")
            nc.vector.reciprocal(sr[:], maxh[:])
            s = small("s")
            nc.vector.tensor_scalar(s[:], sr[:], 127.0, 1e3, op0=ALU.mult, op1=ALU.min)
            nc.vector.tensor_scalar(s[:], s[:], 1e-3, None, op0=ALU.max)
            c1 = small("c1")
            nc.vector.tensor_tensor(c1[:], s[:], rstd[:], op=ALU.mult)
            rs = small("rs")
            nc.vector.reciprocal(rs[:], s[:])
            # q = round(c1 * t1) via magic add/sub (exact RNE), out bf16
            nc.vector.tensor_scalar(t1[:], t1[:], c1[:], MAGIC, op0=ALU.mult, op1=ALU.add)
            q = qp.tile([128, D], dt.bfloat16, tag="q")
            nc.scalar.activation(q[:], t1[:], AF.Identity, bias=nmag[:])
            qT = qtp.tile([128, NDK, 128], dt.bfloat16, tag="qt")
            nc.sync.dma_start_transpose(qT[:], q[:])
            if t == 0:
                _dbg("dbg_q0", q[:], dt.bfloat16, [128, D])
                _dbg("dbg_qt0", qT[:], dt.bfloat16, [128, NDK, 128])
                _dbg("dbg_c1", c1[:], dt.float32, [128, 1])
                _dbg("dbg_rs", rs[:], dt.float32, [128, 1])
            CH[t] = (qT, rs)

        # ---------- up-proj + g quant ----------
        def emit_up(t, bgq=None):
            l, i = divmod(t, n_tok_tiles)
            qT, rs = CH.pop(t)
            wl = W[l]
            g = gp.tile([128, F], dt.float32, tag="g")
            gm8 = scp.tile([128, NFC], dt.float32, tag="gm8")
            for f in range(NFC):
                chk, half = divmod(f, 2)
                hs = slice(half * 512, (half + 1) * 512)
                u_ps = psA.tile([128, 512], dt.float32, tag="ups")
                v_ps = psB.tile([128, 512], dt.float32, tag="vps")
                for dk in range(NDK):
                    nc.tensor.matmul(
                        u_ps[:], qT[:, dk, :], wl["wg"][chk][:, dk, hs],
                        start=(dk == 0), stop=(dk == NDK - 1),
                    )
                    nc.tensor.matmul(
                        v_ps[:], qT[:, dk, :], wl["wu"][chk][:, dk, hs],
                        start=(dk == 0), stop=(dk == NDK - 1),
                    )
                su = silup.tile([128, 512], dt.float32, tag="silu")
                nc.scalar.activation(su[:], u_ps[:], AF.Silu, scale=rs[:])
                nc.vector.tensor_tensor(
                    g[:, f * 512:(f + 1) * 512], su[:], v_ps[:], op=ALU.mult
                )
                nc.vector.tensor_reduce(
                    gm8[:, f:f + 1], g[:, f * 512:(f + 1) * 512],
                    axis=mybir.AxisListType.X, op=ALU.max, apply_absolute_value=True,
                )
                # weave background work (ternarize/wmax) between chunks so no
                # engine queues a monolithic block ahead of critical ops
                if bgq and f % 2 == 1:
                    for _ in range(min(2, len(bgq))):
                        bgq.popleft()()
            if t == 0:
                _dbg("dbg_g0", g[:], dt.float32, [128, F])
            # s2 = clip(127/(max|g|/s + 1e-5)); c2 = s2/s ; rs2 = 1/s2
            gmx = small("gmx")
            nc.vector.tensor_reduce(
                gmx[:], gm8[:], axis=mybir.AxisListType.X, op=ALU.max,
                apply_absolute_value=False,
            )
            nc.vector.tensor_tensor(gmx[:], gmx[:], rs[:], op=ALU.mult)
            nc.vector.tensor_scalar(gmx[:], gmx[:], 1e-5, None, op0=ALU.add)
            s2r = small("s2r")
            nc.vector.reciprocal(s2r[:], gmx[:])
            s2 = small("s2")
            nc.vector.tensor_scalar(s2[:], s2r[:], 127.0, 1e3, op0=ALU.mult, op1=ALU.min)
            nc.vector.tensor_scalar(s2[:], s2[:], 1e-3, None, op0=ALU.max)
            c2 = small("c2")
            nc.vector.tensor_tensor(c2[:], s2[:], rs[:], op=ALU.mult)
            rs2 = small("rs2")
            nc.vector.reciprocal(rs2[:], s2[:])
            # gq = round(c2*g) via magic, out bf16
            nc.vector.tensor_scalar(g[:], g[:], c2[:], MAGIC, op0=ALU.mult, op1=ALU.add)
            gq = gqp.tile([128, F], dt.bfloat16, tag="gq")
            nc.scalar.activation(gq[:], g[:], AF.Identity, bias=nmag[:])
            gqT = gqtp.tile([128, NFT, 128], dt.bfloat16, tag="gqt")
            nc.sync.dma_start_transpose(gqT[:], gq[:])
            if t == 0:
                _dbg("dbg_gq0", gq[:], dt.bfloat16, [128, F])
                _dbg("dbg_s2", s2[:], dt.float32, [128, 1])
                _dbg("dbg_rs2", rs2[:], dt.float32, [128, 1])
            DN[t] = (gqT, rs2)

        # ---------- down-proj + residual ----------
        def emit_down(t):
            l, i = divmod(t, n_tok_tiles)
            gqT, rs2 = DN.pop(t)
            wdl = W[l]["wd"]
            ps0 = psD.tile([128, 512], dt.float32, tag="xdps")
            ps1 = psD.tile([128, 512], dt.float32, tag="xdps")
            for ft in range(NFT):
                qtr, r = divmod(ft, 8)
                nc.tensor.matmul(
                    ps0[:], gqT[:, ft, :], wdl[qtr][:, r, 0:512],
                    start=(ft == 0), stop=(ft == NFT - 1),
                )
                nc.tensor.matmul(
                    ps1[:], gqT[:, ft, :], wdl[qtr][:, r, 512:1024],
                    start=(ft == 0), stop=(ft == NFT - 1),
                )
            for dc, ps in ((0, ps0), (1, ps1)):
                nc.scalar.activation(ps[:], ps[:], AF.Identity, scale=rs2[:])
                nc.vector.tensor_tensor(
                    xres[:, i, dc * 512:(dc + 1) * 512],
                    xres[:, i, dc * 512:(dc + 1) * 512], ps[:], op=ALU.add,
                )
            if l == n_layers - 1:
                nc.sync.dma_start(out_ext[i * 128:(i + 1) * 128, :], xres[:, i, :])

        # ---------- main pipelined loop ----------
        # chain(0) first: its q-transpose must precede the 32 MB of layer-0
        # ternarize loads on the sync ring, else up(0,0) waits ~150 us
        emit_chain(0)
        # layer-0 wg/wu ternarize interleaved with the phase-2 |W|max stream
        # (ph2 reduces ride the DVE queue between tern p1 ops; AllReduce #2
        # completes early in step 0, well before any consumer)
        ph2 = [(l2, mi) for l2 in range(1, n_layers) for mi in range(3)]
        alt = 0
        for ch in range(NCH):
            for ext, key, idx in ((wg_ext, "wg", 0), (wu_ext, "wu", 1)):
                for dk in range(NDK):
                    tern_unit(
                        ext[0, ch, dk], W[0][key][ch][:, dk, :], idx,
                        "act" if alt % 2 else "dve",
                    )
                    if alt % 4 == 1 and ph2:
                        l2, mi = ph2.pop(0)
                        emit_wmax_unit(l2, mi, (slg_ext, slu_ext, sld_ext)[mi])
                    alt += 1
        emit_wmax_allreduce("2", mx_loc2, mx_glob2, 3 * n_layers)
        for _c in range(NCH):
            _dbg(f"dbg_wg{_c}", W[0]["wg"][_c][:], dt.float8e4, [128, NDK, 1024])
            _dbg(f"dbg_wu{_c}", W[0]["wu"][_c][:], dt.float8e4, [128, NDK, 1024])
        tern_sched = {}  # step t -> list of staged-tern units to emit
        ag_sched = {}    # step t -> list of (layer, matrix-key) AllGathers
        GIN, GOUT = {}, {}
        if no_collectives:
            # local full ternarize (sim-debug path)
            for l in range(n_layers - 1):
                make_stage_tiles(l + 1)
                units = tern_units_for_layer(l + 1)
                lo = 2 if l == 0 else 0
                nslots = 7 - lo
                per = (len(units) + nslots - 1) // nslots
                for j, u in enumerate(units):
                    t = l * n_tok_tiles + lo + min(j // per, nslots - 1)
                    tern_sched.setdefault(t, []).append(u)
        else:
            # cooperative ternarize: each core ternarizes only its 1/8
            # row-slice (reusing the slg/slu/sld staging) and per-matrix 4MB
            # AllGathers assemble the fp8 weights during the previous layer.
            for lw in range(1, n_layers):
                GIN[lw] = {k: dram.tile([128, NCH, 1024], dt.float8e4, tag=f"gin_{k}", name=f"gin_{k}")
                           for k in ("wg", "wu", "wd")}
                GOUT[lw] = {k: dram.tile([n_cores, 128, NCH, 1024], dt.float8e4, tag=f"gout_{k}", name=f"gout_{k}", addr_space="Shared")
                            for k in ("wg", "wu", "wd")}
                units = []
                for mi, (sl, k) in enumerate(((slg_ext, "wg"), (slu_ext, "wu"), (sld_ext, "wd"))):
                    for ch in range(NCH):
                        units.append((sl[lw, ch], GIN[lw][k][:, ch, :], 3 * lw + mi))
                t0 = (lw - 1) * n_tok_tiles + (1 if lw == 1 else 0)
                for j, u in enumerate(units):
                    tern_sched.setdefault(t0 + j // 6, []).append(u)
                for j, k in enumerate(("wg", "wu", "wd")):
                    ag_sched.setdefault(t0 + 2 + j, []).append((lw, k))

        from collections import deque
        for t in range(nsteps):
            l, i = divmod(t, n_tok_tiles)
            # background work for this step, woven between up-proj chunks:
            # ternarize units cycle dve/act (every 7th on idle gpsimd), the
            # phase-2 wmax stream rides along in steps 0-1.
            # dve-mode units weave between up-proj chunks (DVE has slack
            # there); act-mode (AF.Sign p2) go to the step tail so they never
            # queue ahead of the silu/round ops on the Act FIFO.
            bgq = deque()
            bgtail = deque()
            if t == 0:
                for j, (src_ap, dst_ap) in enumerate(l0_wd_units):
                    if j % 2:
                        bgtail.append(lambda a=src_ap, b=dst_ap: tern_unit(a, b, 2, "act"))
                    else:
                        bgq.append(lambda a=src_ap, b=dst_ap: tern_unit(a, b, 2, "dve"))
            for j, (src_ap, stage_ap, idx) in enumerate(tern_sched.get(t, ())):
                if j % 2:
                    bgtail.append(lambda a=src_ap, b=stage_ap, x=idx: tern_unit_staged(a, b, x, "act"))
                else:
                    bgq.append(lambda a=src_ap, b=stage_ap, x=idx: tern_unit_staged(a, b, x, "dve"))

            # layer-boundary: rms scale broadcast for next layer (before chain)
            if i == n_tok_tiles - 1 and l + 1 < n_layers:
                emit_scale_bc(l + 1)
            if t + 1 < nsteps:
                emit_chain(t + 1)
            emit_up(t, bgq)
            if i == n_tok_tiles - 1 and l + 1 < n_layers:
                # wg/wu refills for next layer; emitted after their last reads
                W[l + 1] = {
                    "wg": [wts.tile([128, NDK, 1024], dt.float8e4, tag=f"wg{ch}", name=f"wg{ch}") for ch in range(NCH)],
                    "wu": [wts.tile([128, NDK, 1024], dt.float8e4, tag=f"wu{ch}", name=f"wu{ch}") for ch in range(NCH)],
                }
                for ch in range(NCH):
                    if no_collectives:
                        nc.sync.dma_start(W[l + 1]["wg"][ch][:], STG[l + 1]["wg"][ch][:])
                        nc.sync.dma_start(W[l + 1]["wu"][ch][:], STG[l + 1]["wu"][ch][:])
                    else:
                        for dk in range(NDK):
                            nc.sync.dma_start(W[l + 1]["wg"][ch][:, dk, :], GOUT[l + 1]["wg"][dk, :, ch, :])
                            nc.sync.dma_start(W[l + 1]["wu"][ch][:, dk, :], GOUT[l + 1]["wu"][dk, :, ch, :])
            if t >= 1:
                emit_down(t - 1)
            for lw, k in ag_sched.get(t, ()):
                nc.gpsimd.collective_compute(
                    "AllGather",
                    ALU.bypass,
                    replica_groups=[list(range(n_cores))],
                    ins=[GIN[lw][k][:].opt()],
                    outs=[GOUT[lw][k][:].opt()],
                )
            if i == 0 and l >= 1:
                # wd refill for this layer; after down(l-1, 7) was emitted
                W[l]["wd"] = [wts.tile([128, 8, 1024], dt.float8e4, tag=f"wd{q}", name=f"wd{q}") for q in range(NCH)]
                for q_ in range(NCH):
                    if no_collectives:
                        nc.gpsimd.dma_start(W[l]["wd"][q_][:], STG[l]["wd"][q_][:])
                    else:
                        nc.gpsimd.dma_start(W[l]["wd"][q_][:, 0:4, :], GOUT[l]["wd"][2 * q_, :, :, :])
                        nc.gpsimd.dma_start(W[l]["wd"][q_][:, 4:8, :], GOUT[l]["wd"][2 * q_ + 1, :, :, :])
            if t == 0:
                # wsc2 broadcast here: the PE reaches it right as AllReduce #2
                # lands, without head-blocking the prologue matmuls
                wsc2_box[0] = emit_wmax_bcast("2", mx_glob2)
            # drain leftover background work
            while bgq:
                bgq.popleft()()
            while bgtail:
                bgtail.popleft()()
            if t == 0:
                _dbg("dbg_wd0", W[0]["wd"][0][:], dt.float8e4, [128, 8, 1024])

        emit_down(nsteps - 1)

    if not no_collectives:  # sim-debug mode chokes on the injected waits
        _split_excess_waits(nc)
    return nc


_nc_cache = {}


def _get_nc(key=(NCORES, NTT, L)):
    if key not in _nc_cache:
        _nc_cache[key] = build(*key)
    return _nc_cache[key]


def _repack(w, n_layers):
    # [L, D, F] -> [L, F//1024, D//128, 128, 1024] so each (ch, dk) tile is
    # contiguous; dk == core index for the per-core max slices.
    return np.ascontiguousarray(
        w.reshape(n_layers, D // 128, 128, F // 1024, 1024).transpose(0, 3, 1, 2, 4)
    )


def _make_in_maps(x, rs, wg, wu, wd, n_cores=NCORES):
    n_layers = rs.shape[0]
    wg_r = _repack(wg, n_layers)
    wu_r = _repack(wu, n_layers)
    nsl = NDK // n_cores  # dk-slabs per core for slg/slu
    dsl = F // n_cores
    in_maps = []
    for c in range(n_cores):
        slg = wg_r[:, :, c * nsl:(c + 1) * nsl].reshape(n_layers, -1, 128, 1024)
        slu = wu_r[:, :, c * nsl:(c + 1) * nsl].reshape(n_layers, -1, 128, 1024)
        sld = wd[:, c * dsl:(c + 1) * dsl, :].reshape(n_layers, -1, 128, 1024)
        in_maps.append({
            "x": x[c],
            "rs": rs,
            "wg": wg_r,
            "wu": wu_r,
            "wd": wd,
            "slg": np.ascontiguousarray(slg),
            "slu": np.ascontiguousarray(slu),
            "sld": np.ascontiguousarray(sld),
        })
    return in_maps


def kernel(x, rms_scale, W_g, W_u, W_d):
    """Full-input entry point: shard over batch, run 8-core SPMD, gather."""
    x = np.ascontiguousarray(np.asarray(x, dtype=np.float32))
    rs = np.ascontiguousarray(np.asarray(rms_scale, dtype=np.float32))
    wg = np.ascontiguousarray(np.asarray(W_g, dtype=np.float32))
    wu = np.ascontiguousarray(np.asarray(W_u, dtype=np.float32))
    wd = np.ascontiguousarray(np.asarray(W_d, dtype=np.float32))
    B, S, Dx = x.shape
    assert (B, S, Dx) == (NCORES, TOK, D), (B, S, Dx)
    nc = _get_nc()
    in_maps = _make_in_maps(x, rs, wg, wu, wd)
    res = run_bass_kernel_spmd(nc, in_maps, list(range(NCORES)))
    return np.stack([res.results[c]["out"] for c in range(NCORES)], axis=0)


# revision 53
# speedup vs baseline: 1.1330x; 1.1330x over previous
"""Trainium2 Bass kernel for nn_CascadeTransformerMM (4-layer ternary-GLU cascade).

Math (per layer, per token row):
  h   = rms_scale * x * rsqrt(mean(x^2) + 1e-6)
  s   = clip(127/(max|h| + 1e-5), 1e-3, 1e3);  q = round(s*h)      (ints in [-127,127])
  Wt  = clip(round(W * 127/(max|W| + 1e-5)), -1, 1)                 (ternary {-1,0,1})
  u   = (q @ Wg_t)/s ; v = (q @ Wu_t)/s ; g = silu(u)*v
  s2  = clip(127/(max|g| + 1e-5), 1e-3, 1e3); gq = round(s2*g)
  x  += (gq @ Wd_t)/s2

Distribution: pure data-parallel over the batch dim (8 batches -> 8 cores),
weights replicated per core. Per-matrix |W|max is computed cooperatively:
each core reduces a 1/8 row-slice, then two tiny AllReduce(max) ops (layer 0
first so ternarization can start early; layers 1-3 overlapped with layer-0
compute) share the 12 scalars.

Schedule: x stays SBUF-resident for all 4 layers. One software-pipelined
stream of 32 tile-steps; step t emits up-proj(t) then down-proj(t-1) on the
PE so the g-quant + transpose latency of tile t hides under other matmuls.
Stats+quant for tile t+1 run in the up-proj(t) shadow. Weight ternarization
(fp32 -> fp8 {-1,0,1}) runs on the otherwise-idle GPSIMD engine, spread
across the steps of the previous layer, staged via DRAM; SBUF weight tiles
are chunked so boundary refills overlap the previous layer's tail."""

import os
import sys

for _p in ("/opt/trn_rl_repo", "/root/.axon_site/_ro/trn_rl_repo"):
    if os.path.isdir(_p) and _p not in sys.path:
        sys.path.insert(0, _p)

import numpy as np
from contextlib import ExitStack

import concourse.bass as bass
import concourse.mybir as mybir
import concourse.tile as tile
from concourse.bass_utils import run_bass_kernel_spmd

dt = mybir.dt
AF = mybir.ActivationFunctionType
ALU = mybir.AluOpType

MAGIC = float(1.5 * 2**23)  # fp32 round-to-nearest-even magic constant
D = 1024
F = 4096
L = 4
NCORES = 8
TOK = 1024  # tokens per core (one batch of S=1024)

NDK = D // 128   # 8 contraction tiles for up-proj
NFT = F // 128   # 32 contraction tiles for down-proj
NFC = F // 512   # 8 free-dim chunks for up-proj
NTT = TOK // 128  # 8 token tiles
NCH = F // 1024  # 4 weight chunks (wg/wu) / quarters (wd)


def _split_excess_waits(nc, max_waits: int = 1) -> int:
    """walrus in this container rejects >1 sync-wait per instruction; split
    extras into standalone event-semaphore waits on the same engine (same-
    engine program order makes this semantically identical)."""
    n = 0
    for func in nc.m.functions:
        for block in func.blocks:
            changed = False
            out = []
            for inst in block.instructions:
                si = getattr(inst, "sync_info", None)
                if si is not None and si.on_wait and len(si.on_wait) > max_waits:
                    waits = list(si.on_wait)
                    for j, w in enumerate(waits[max_waits:]):
                        out.append(
                            mybir.InstEventSemaphore(
                                name=f"{inst.name}-xw{j}",
                                engine=inst.engine,
                                ins=[],
                                outs=[],
                                sync_info=mybir.SyncInfo(on_wait=[w], on_update=[]),
                            )
                        )
                        n += 1
                    inst.sync_info = mybir.SyncInfo(
                        on_wait=waits[:max_waits], on_update=list(si.on_update)
                    )
                    changed = True
                out.append(inst)
            if changed:
                block.instructions = out
    return n


def build(n_cores: int = NCORES, n_tok_tiles: int = NTT, n_layers: int = L,
          no_collectives: bool = False, debug_dump: bool = False) -> bass.Bass:
    nc = bass.Bass(num_devices=n_cores)
    _dbg_tensors = {}

    def _dbg(name, ap, dtype, shape):
        if not debug_dump:
            return
        d = nc.dram_tensor(name, shape, dtype)
        _dbg_tensors[name] = d
        nc.sync.dma_start(d[:], ap)
    tok = n_tok_tiles * 128

    x_ext = nc.declare_dram_parameter("x", [tok, D], dt.float32, isOutput=False)
    rs_ext = nc.declare_dram_parameter("rs", [n_layers, D], dt.float32, isOutput=False)
    # wg/wu repacked host-side to [L, F//1024, NDK, 128, 1024] so every
    # [128, 1024] weight tile is one contiguous 512 KB DMA; wd's row-slabs
    # are naturally contiguous.
    wg_ext = nc.declare_dram_parameter("wg", [n_layers, NCH, NDK, 128, 1024], dt.float32, isOutput=False)
    wu_ext = nc.declare_dram_parameter("wu", [n_layers, NCH, NDK, 128, 1024], dt.float32, isOutput=False)
    wd_ext = nc.declare_dram_parameter("wd", [n_layers, F, D], dt.float32, isOutput=False)
    # per-core row-slices of each matrix for the cooperative |W|max,
    # as [L, nun, 128, 1024] contiguous units
    nun = (D // n_cores) * F // (128 * 1024)
    slg_ext = nc.declare_dram_parameter("slg", [n_layers, nun, 128, 1024], dt.float32, isOutput=False)
    slu_ext = nc.declare_dram_parameter("slu", [n_layers, nun, 128, 1024], dt.float32, isOutput=False)
    sld_ext = nc.declare_dram_parameter("sld", [n_layers, nun, 128, 1024], dt.float32, isOutput=False)
    out_ext = nc.declare_dram_parameter("out", [tok, D], dt.float32, isOutput=True)

    mx_loc1 = nc.dram_tensor("mx_loc1", [1, 16], dt.float32)
    mx_glob1 = nc.dram_tensor("mx_glob1", [1, 16], dt.float32)
    mx_loc2 = nc.dram_tensor("mx_loc2", [1, 16], dt.float32)
    mx_glob2 = nc.dram_tensor("mx_glob2", [1, 16], dt.float32)

    nsteps = n_layers * n_tok_tiles

    with tile.TileContext(nc) as tc, ExitStack() as ctx:
        P = ctx.enter_context
        wts = P(tc.tile_pool(name="wts", bufs=1))       # fp8 weight chunk tiles
        xp = P(tc.tile_pool(name="xres", bufs=1))
        gp = P(tc.tile_pool(name="g", bufs=1))
        gqp = P(tc.tile_pool(name="gq", bufs=1))
        gqtp = P(tc.tile_pool(name="gqt", bufs=2))
        t1p = P(tc.tile_pool(name="t1", bufs=1))
        qp = P(tc.tile_pool(name="q", bufs=1))
        qtp = P(tc.tile_pool(name="qt", bufs=2))
        silup = P(tc.tile_pool(name="silu", bufs=1))
        wsp = P(tc.tile_pool(name="wstream", bufs=2))
        wmp = P(tc.tile_pool(name="wmstream", bufs=2))
        wi8p = P(tc.tile_pool(name="wi8", bufs=2))
        w8sp = P(tc.tile_pool(name="w8s", bufs=2))
        scbp = P(tc.tile_pool(name="scb", bufs=1))
        scp = P(tc.tile_pool(name="sc", bufs=4))
        constp = P(tc.tile_pool(name="const", bufs=1))
        dram = P(tc.tile_pool(name="dram", bufs=2, space="DRAM"))
        psA = P(tc.tile_pool(name="psA", bufs=2, space="PSUM"))
        psB = P(tc.tile_pool(name="psB", bufs=2, space="PSUM"))
        psD = P(tc.tile_pool(name="psD", bufs=4, space="PSUM"))

        # ---------- constants ----------
        ones1 = constp.tile([1, 128], dt.float32, tag="ones1")
        nc.gpsimd.memset(ones1[:], 1.0)
        mag = constp.tile([128, 1], dt.float32, tag="mag")
        nc.gpsimd.memset(mag[:], MAGIC)
        nmag = constp.tile([128, 1], dt.float32, tag="nmag")
        nc.gpsimd.memset(nmag[:], -MAGIC)

        def small(tag):
            return scp.tile([128, 1], dt.float32, tag=tag, name=tag)

        # ---------- cooperative per-matrix |W|max (two phases) ----------
        wmax_cols = constp.tile([128, 16], dt.float32, tag="wmaxc")
        nc.gpsimd.memset(wmax_cols[:], 0.0)

        _wm_alt = [0]

        def emit_wmax_unit(l, mi, ext):
            idx = 3 * l + mi
            part = scp.tile([128, nun], dt.float32, tag="wmaxpart")
            for un in range(nun):
                wt = wmp.tile([128, 1024], dt.float32, tag="wmstream")
                ring = nc.gpsimd if _wm_alt[0] % 2 == 0 else nc.scalar
                _wm_alt[0] += 1
                ring.dma_start(wt[:], ext[l, un])
                nc.vector.tensor_reduce(
                    part[:, un:un + 1], wt[:], axis=mybir.AxisListType.X,
                    op=ALU.max, apply_absolute_value=True,
                )
            nc.vector.tensor_reduce(
                wmax_cols[:, idx:idx + 1], part[:, 0:nun],
                axis=mybir.AxisListType.X, op=ALU.max,
                apply_absolute_value=False,
            )

        def emit_wmax_allreduce(tag, mloc, mglob, ncols):
            mrow = constp.tile([1, 16], dt.float32, tag=f"mrow{tag}")
            nc.gpsimd.memset(mrow[:], 0.0)
            nc.gpsimd.tensor_reduce(
                mrow[:, 0:ncols], wmax_cols[:, 0:ncols], axis=mybir.AxisListType.C,
                op=ALU.max,
            )
            nc.sync.dma_start(mloc[:], mrow[:])
            if no_collectives:
                # sim-debug mode: local max stands in for the global one
                nc.sync.dma_start(mglob[:], mloc[:])
            else:
                nc.gpsimd.collective_compute(
                    "AllReduce",
                    ALU.max,
                    replica_groups=[list(range(n_cores))],
                    ins=[mloc[:].opt()],
                    outs=[mglob[:].opt()],
                )

        def emit_wmax_bcast(tag, mglob):
            grow = constp.tile([1, 16], dt.float32, tag=f"grow{tag}")
            nc.sync.dma_start(grow[:], mglob[:])
            bc = psD.tile([128, 512], dt.float32, tag="xdps")
            nc.tensor.matmul(bc[:, 0:16], ones1[:], grow[:], start=True, stop=True)
            msc = constp.tile([128, 16], dt.float32, tag=f"msc{tag}")
            nc.vector.tensor_scalar(msc[:], bc[:, 0:16], 1e-5, None, op0=ALU.add)
            mrec = constp.tile([128, 16], dt.float32, tag=f"mrec{tag}")
            nc.vector.reciprocal(mrec[:], msc[:])
            wsc = constp.tile([128, 16], dt.float32, tag=f"wsc{tag}")
            nc.vector.tensor_scalar(wsc[:], mrec[:], 127.0, None, op0=ALU.mult)
            return wsc

        # phase 1: layer 0 only (3 scalars) -> ternarize layer 0 can start early
        for mi, ext in enumerate((slg_ext, slu_ext, sld_ext)):
            emit_wmax_unit(0, mi, ext)
        emit_wmax_allreduce("1", mx_loc1, mx_glob1, 3)
        wsc1 = emit_wmax_bcast("1", mx_glob1)
        # phase 2 (layers 1-3) is spread over steps 0-2 to keep the gpsimd
        # queue free for layer-0 ternarize; AllReduce #2 lands in step 2.
        wsc2_box = [None]

        def wsc_ap(idx):
            return (wsc1 if idx < 3 else wsc2_box[0])[:, idx:idx + 1]

        # ---------- x resident load ----------
        xres = xp.tile([128, n_tok_tiles, D], dt.float32, tag="xres")
        for i in range(n_tok_tiles):
            nc.sync.dma_start(xres[:, i, :], x_ext[i * 128:(i + 1) * 128, :])

        # ---------- scale_bc for layer 0 ----------
        SCB = {}

        def emit_scale_bc(l):
            rrow = t1p.tile([128, D], dt.float32, tag="t1")
            nc.sync.dma_start(rrow[0:1, :], rs_ext[l:l + 1, :])
            scb = scbp.tile([128, D], dt.float32, tag="scb")
            for h in range(2):
                bc = psD.tile([128, 512], dt.float32, tag="xdps")
                nc.tensor.matmul(
                    bc[:], ones1[:], rrow[0:1, h * 512:(h + 1) * 512],
                    start=True, stop=True,
                )
                nc.scalar.activation(scb[:, h * 512:(h + 1) * 512], bc[:], AF.Identity)
            SCB[l] = scb

        emit_scale_bc(0)

        # ---------- weight tiles / ternarize ----------
        # W[l] = {"wg": [4 tiles of [128, NDK, 1024]], "wu": ..., "wd": [4 tiles of [128, 8, 1024]]}
        W = {}
        STG = {}  # DRAM fp8 staging for layers >= 1

        def tern_unit(src_ap, dst_ap, idx, mode):
            # p1 fp32 -> int8 convert (RNE, values <= 127 so exact); p2 is
            # clip(int,-1,1) == sign(int). Engine per `mode`: 'dve' both on
            # DVE (min/max), 'act' p1 DVE + p2 Act AF.Sign, 'gpsimd' both on
            # the (slow but otherwise idle) gpsimd.
            wt = wsp.tile([128, 1024], dt.float32, tag="wstream")
            nc.sync.dma_start(wt[:], src_ap)
            w8i = wi8p.tile([128, 1024], dt.int8, tag="wi8")
            if mode == "gpsimd":
                nc.gpsimd.tensor_scalar(w8i[:], wt[:], wsc_ap(idx), None, op0=ALU.mult)
                nc.gpsimd.tensor_scalar(dst_ap, w8i[:], 1, -1, op0=ALU.min, op1=ALU.max)
            else:
                nc.vector.tensor_scalar(w8i[:], wt[:], wsc_ap(idx), None, op0=ALU.mult)
                if mode == "act":
                    nc.scalar.activation(dst_ap, w8i[:], AF.Sign)
                else:
                    nc.vector.tensor_scalar(dst_ap, w8i[:], 1, -1, op0=ALU.min, op1=ALU.max)

        def tern_unit_staged(src_ap, stage_ap, idx, mode):
            stg = w8sp.tile([128, 1024], dt.float8e4, tag="w8s")
            tern_unit(src_ap, stg[:], idx, mode)
            nc.gpsimd.dma_start(stage_ap, stg[:])

        # layer 0: direct-to-SBUF. wg/wu on gpsimd (loads on sync ring).
        W[0] = {
            "wg": wts.tile([128, NDK, F], dt.float8e4, tag="wg", name="wg"),
            "wu": wts.tile([128, NDK, F], dt.float8e4, tag="wu", name="wu"),
            "wd": wts.tile([128, NFT, 1024], dt.float8e4, tag="wd", name="wd"),
        }
        # layer-0 wd deferred into steps 0-1 (DVE+Act, loads on scalar ring)
        l0_wd_units = [
            (wd_ext[0, ft * 128:(ft + 1) * 128, :], W[0]["wd"][:, ft, :])
            for ft in range(NFT)
        ]

        # tern work lists for layers >= 1 (units spread over prev layer's steps)
        def make_stage_tiles(l):
            STG[l] = {
                "wg": [dram.tile([128, NDK, 1024], dt.float8e4, tag=f"swg{ch}", name=f"swg{ch}") for ch in range(NCH)],
                "wu": [dram.tile([128, NDK, 1024], dt.float8e4, tag=f"swu{ch}", name=f"swu{ch}") for ch in range(NCH)],
                "wd": [dram.tile([128, 8, 1024], dt.float8e4, tag=f"swd{q}", name=f"swd{q}") for q in range(NCH)],
            }

        def tern_units_for_layer(l):
            units = []
            for ch in range(NCH):
                for ext, key, mi in ((wg_ext, "wg", 0), (wu_ext, "wu", 1)):
                    for dk in range(NDK):
                        units.append((ext[l, ch, dk], STG[l][key][ch][:, dk, :], 3 * l + mi))
            for ft in range(NFT):
                units.append((
                    wd_ext[l, ft * 128:(ft + 1) * 128, :],
                    STG[l]["wd"][ft // 8][:, ft % 8, :],
                    3 * l + 2,
                ))
            return units

        # ---------- per-tile stats + activation quant chain ----------
        CH = {}   # t -> (qT, rs)
        DN = {}   # t -> (gqT, rs2)

        def emit_chain(t):
            l, i = divmod(t, n_tok_tiles)
            xsl = xres[:, i, :]
            scb = SCB[l]
            t1 = t1p.tile([128, D], dt.float32, tag="t1")
            ssq = small("ssq")
            nc.scalar.activation(t1[:], xsl, AF.Square, accum_out=ssq[:])
            nc.vector.tensor_tensor(t1[:], xsl, scb[:], op=ALU.mult)
            mx = small("mx")
            nc.vector.tensor_reduce(
                mx[:], t1[:], axis=mybir.AxisListType.X, op=ALU.max,
                apply_absolute_value=True,
            )
            ms = small("ms")
            nc.vector.tensor_scalar(ms[:], ssq[:], 1.0 / D, 1e-6, op0=ALU.mult, op1=ALU.add)
            rt = small("rt")
            nc.scalar.activation(rt[:], ms[:], AF.Sqrt)
            rstd = small("rstd")
            nc.vector.reciprocal(rstd[:], rt[:])
            # one Newton step: rstd *= 1.5 - 0.5*ms*rstd^2  (fixes the ~7e-6
            # Sqrt-LUT error that quantization tie-flips amplify layer by layer)
            nwt = small("nwt")
            nc.vector.tensor_tensor(nwt[:], rstd[:], rstd[:], op=ALU.mult)
            nc.vector.tensor_tensor(nwt[:], nwt[:], ms[:], op=ALU.mult)
            nc.vector.tensor_scalar(nwt[:], nwt[:], -0.5, 1.5, op0=ALU.mult, op1=ALU.add)
            nc.vector.tensor_tensor(rstd[:], rstd[:], nwt[:], op=ALU.mult)
            maxh = small("maxh")
            nc.vector.tensor_tensor(maxh[:], mx[:], rstd[:], op=ALU.mult)
            nc.vector.tensor_scalar(maxh[:], maxh[:], 1e-5, None, op0=ALU.add)
            sr = small("sr")
            nc.vector.reciprocal(sr[:], maxh[:])
            s = small("s")
            nc.vector.tensor_scalar(s[:], sr[:], 127.0, 1e3, op0=ALU.mult, op1=ALU.min)
            nc.vector.tensor_scalar(s[:], s[:], 1e-3, None, op0=ALU.max)
            c1 = small("c1")
            nc.vector.tensor_tensor(c1[:], s[:], rstd[:], op=ALU.mult)
            rs = small("rs")
            nc.vector.reciprocal(rs[:], s[:])
            # q = round(c1 * t1) via magic add/sub (exact RNE), out bf16
            nc.vector.tensor_scalar(t1[:], t1[:], c1[:], MAGIC, op0=ALU.mult, op1=ALU.add)
            q = qp.tile([128, D], dt.bfloat16, tag="q")
            nc.scalar.activation(q[:], t1[:], AF.Identity, bias=nmag[:])
            qT = qtp.tile([128, NDK, 128], dt.bfloat16, tag="qt")
            nc.sync.dma_start_transpose(qT[:], q[:])
            if t == 0:
                _dbg("dbg_q0", q[:], dt.bfloat16, [128, D])
                _dbg("dbg_qt0", qT[:], dt.bfloat16, [128, NDK, 128])
                _dbg("dbg_c1", c1[:], dt.float32, [128, 1])
                _dbg("dbg_rs", rs[:], dt.float32, [128, 1])
            CH[t] = (qT, rs)

        # ---------- up-proj + g quant ----------
        def emit_up(t, bgq=None):
            l, i = divmod(t, n_tok_tiles)
            qT, rs = CH.pop(t)
            wl = W[l]
            g = gp.tile([128, F], dt.float32, tag="g")
            gm8 = scp.tile([128, NFC], dt.float32, tag="gm8")
            for f in range(NFC):
                chk, half = divmod(f, 2)
                hs = slice(half * 512, (half + 1) * 512)
                u_ps = psA.tile([128, 512], dt.float32, tag="ups")
                v_ps = psB.tile([128, 512], dt.float32, tag="vps")
                fs = slice(f * 512, (f + 1) * 512)
                for dk in range(NDK):
                    nc.tensor.matmul(
                        u_ps[:], qT[:, dk, :], wl["wg"][:, dk, fs],
                        start=(dk == 0), stop=(dk == NDK - 1),
                    )
                    nc.tensor.matmul(
                        v_ps[:], qT[:, dk, :], wl["wu"][:, dk, fs],
                        start=(dk == 0), stop=(dk == NDK - 1),
                    )
                su = silup.tile([128, 512], dt.float32, tag="silu")
                nc.scalar.activation(su[:], u_ps[:], AF.Silu, scale=rs[:])
                nc.vector.tensor_tensor(
                    g[:, f * 512:(f + 1) * 512], su[:], v_ps[:], op=ALU.mult
                )
                nc.vector.tensor_reduce(
                    gm8[:, f:f + 1], g[:, f * 512:(f + 1) * 512],
                    axis=mybir.AxisListType.X, op=ALU.max, apply_absolute_value=True,
                )
                # weave background work (ternarize/wmax) between chunks so no
                # engine queues a monolithic block ahead of critical ops
                if bgq and f % 2 == 1:
                    for _ in range(min(2, len(bgq))):
                        bgq.popleft()()
            if t == 0:
                _dbg("dbg_g0", g[:], dt.float32, [128, F])
            # s2 = clip(127/(max|g|/s + 1e-5)); c2 = s2/s ; rs2 = 1/s2
            gmx = small("gmx")
            nc.vector.tensor_reduce(
                gmx[:], gm8[:], axis=mybir.AxisListType.X, op=ALU.max,
                apply_absolute_value=False,
            )
            nc.vector.tensor_tensor(gmx[:], gmx[:], rs[:], op=ALU.mult)
            nc.vector.tensor_scalar(gmx[:], gmx[:], 1e-5, None, op0=ALU.add)
            s2r = small("s2r")
            nc.vector.reciprocal(s2r[:], gmx[:])
            s2 = small("s2")
            nc.vector.tensor_scalar(s2[:], s2r[:], 127.0, 1e3, op0=ALU.mult, op1=ALU.min)
            nc.vector.tensor_scalar(s2[:], s2[:], 1e-3, None, op0=ALU.max)
            c2 = small("c2")
            nc.vector.tensor_tensor(c2[:], s2[:], rs[:], op=ALU.mult)
            rs2 = small("rs2")
            nc.vector.reciprocal(rs2[:], s2[:])
            # gq = round(c2*g) via magic, out bf16
            nc.vector.tensor_scalar(g[:], g[:], c2[:], MAGIC, op0=ALU.mult, op1=ALU.add)
            gq = gqp.tile([128, F], dt.bfloat16, tag="gq")
            nc.scalar.activation(gq[:], g[:], AF.Identity, bias=nmag[:])
            gqT = gqtp.tile([128, NFT, 128], dt.bfloat16, tag="gqt")
            nc.sync.dma_start_transpose(gqT[:], gq[:])
            if t == 0:
                _dbg("dbg_gq0", gq[:], dt.bfloat16, [128, F])
                _dbg("dbg_s2", s2[:], dt.float32, [128, 1])
                _dbg("dbg_rs2", rs2[:], dt.float32, [128, 1])
            DN[t] = (gqT, rs2)

        # ---------- down-proj + residual ----------
        def emit_down(t):
            l, i = divmod(t, n_tok_tiles)
            gqT, rs2 = DN.pop(t)
            wdl = W[l]["wd"]
            ps0 = psD.tile([128, 512], dt.float32, tag="xdps")
            ps1 = psD.tile([128, 512], dt.float32, tag="xdps")
            for ft in range(NFT):
                nc.tensor.matmul(
                    ps0[:], gqT[:, ft, :], wdl[:, ft, 0:512],
                    start=(ft == 0), stop=(ft == NFT - 1),
                )
                nc.tensor.matmul(
                    ps1[:], gqT[:, ft, :], wdl[:, ft, 512:1024],
                    start=(ft == 0), stop=(ft == NFT - 1),
                )
            for dc, ps in ((0, ps0), (1, ps1)):
                nc.scalar.activation(ps[:], ps[:], AF.Identity, scale=rs2[:])
                nc.vector.tensor_tensor(
                    xres[:, i, dc * 512:(dc + 1) * 512],
                    xres[:, i, dc * 512:(dc + 1) * 512], ps[:], op=ALU.add,
                )
            if l == n_layers - 1:
                nc.sync.dma_start(out_ext[i * 128:(i + 1) * 128, :], xres[:, i, :])

        # ---------- main pipelined loop ----------
        # chain(0) first: its q-transpose must precede the 32 MB of layer-0
        # ternarize loads on the sync ring, else up(0,0) waits ~150 us
        emit_chain(0)
        # layer-0 wg/wu ternarize interleaved with the phase-2 |W|max stream
        # (ph2 reduces ride the DVE queue between tern p1 ops; AllReduce #2
        # completes early in step 0, well before any consumer)
        ph2 = [(l2, mi) for l2 in range(1, n_layers) for mi in range(3)]
        alt = 0
        for ch in range(NCH):
            for ext, key, idx in ((wg_ext, "wg", 0), (wu_ext, "wu", 1)):
                for dk in range(NDK):
                    tern_unit(
                        ext[0, ch, dk], W[0][key][:, dk, ch * 1024:(ch + 1) * 1024], idx,
                        "act" if alt % 2 else "dve",
                    )
                    if alt % 4 == 1 and ph2:
                        l2, mi = ph2.pop(0)
                        emit_wmax_unit(l2, mi, (slg_ext, slu_ext, sld_ext)[mi])
                    alt += 1
        emit_wmax_allreduce("2", mx_loc2, mx_glob2, 3 * n_layers)
        tern_sched = {}  # step t -> list of staged-tern units to emit
        ag_sched = {}    # step t -> list of (layer, matrix-key) AllGathers
        GIN, GOUT = {}, {}
        if no_collectives:
            # local full ternarize (sim-debug path)
            for l in range(n_layers - 1):
                make_stage_tiles(l + 1)
                units = tern_units_for_layer(l + 1)
                lo = 2 if l == 0 else 0
                nslots = 7 - lo
                per = (len(units) + nslots - 1) // nslots
                for j, u in enumerate(units):
                    t = l * n_tok_tiles + lo + min(j // per, nslots - 1)
                    tern_sched.setdefault(t, []).append(u)
        else:
            # cooperative ternarize: each core ternarizes only its 1/8
            # row-slice (reusing the slg/slu/sld staging) and per-matrix 4MB
            # AllGathers assemble the fp8 weights during the previous layer.
            for lw in range(1, n_layers):
                GIN[lw] = {k: dram.tile([128, NCH, 1024], dt.float8e4, tag=f"gin_{k}", name=f"gin_{k}")
                           for k in ("wg", "wu", "wd")}
                GOUT[lw] = {k: dram.tile([n_cores, 128, NCH, 1024], dt.float8e4, tag=f"gout_{k}", name=f"gout_{k}", addr_space="Shared")
                            for k in ("wg", "wu", "wd")}
                units = []
                for mi, (sl, k) in enumerate(((slg_ext, "wg"), (slu_ext, "wu"), (sld_ext, "wd"))):
                    for ch in range(NCH):
                        units.append((sl[lw, ch], GIN[lw][k][:, ch, :], 3 * lw + mi))
                t0 = (lw - 1) * n_tok_tiles + (1 if lw == 1 else 0)
                for j, u in enumerate(units):
                    tern_sched.setdefault(t0 + j // 6, []).append(u)
                for k in ("wg", "wu", "wd"):
                    ag_sched.setdefault(t0 + 2, []).append((lw, k))

        from collections import deque
        for t in range(nsteps):
            l, i = divmod(t, n_tok_tiles)
            # background work for this step, woven between up-proj chunks:
            # ternarize units cycle dve/act (every 7th on idle gpsimd), the
            # phase-2 wmax stream rides along in steps 0-1.
            # dve-mode units weave between up-proj chunks (DVE has slack
            # there); act-mode (AF.Sign p2) go to the step tail so they never
            # queue ahead of the silu/round ops on the Act FIFO.
            bgq = deque()
            bgtail = deque()
            if t == 0:
                for j, (src_ap, dst_ap) in enumerate(l0_wd_units):
                    if j % 2:
                        bgtail.append(lambda a=src_ap, b=dst_ap: tern_unit(a, b, 2, "act"))
                    else:
                        bgq.append(lambda a=src_ap, b=dst_ap: tern_unit(a, b, 2, "dve"))
            for j, (src_ap, stage_ap, idx) in enumerate(tern_sched.get(t, ())):
                if j % 2:
                    bgtail.append(lambda a=src_ap, b=stage_ap, x=idx: tern_unit_staged(a, b, x, "act"))
                else:
                    bgq.append(lambda a=src_ap, b=stage_ap, x=idx: tern_unit_staged(a, b, x, "dve"))

            # layer-boundary: rms scale broadcast for next layer (before chain)
            if i == n_tok_tiles - 1 and l + 1 < n_layers:
                emit_scale_bc(l + 1)
            if t + 1 < nsteps:
                emit_chain(t + 1)
            emit_up(t, bgq)
            if i == n_tok_tiles - 1 and l + 1 < n_layers:
                # wg/wu refills for next layer; emitted after their last reads
                W[l + 1] = {
                    "wg": wts.tile([128, NDK, F], dt.float8e4, tag="wg", name="wg"),
                    "wu": wts.tile([128, NDK, F], dt.float8e4, tag="wu", name="wu"),
                }
                if no_collectives:
                    for ch in range(NCH):
                        nc.sync.dma_start(W[l + 1]["wg"][:, :, ch * 1024:(ch + 1) * 1024], STG[l + 1]["wg"][ch][:])
                        nc.sync.dma_start(W[l + 1]["wu"][:, :, ch * 1024:(ch + 1) * 1024], STG[l + 1]["wu"][ch][:])
                else:
                    for dk in range(NDK):
                        nc.sync.dma_start(W[l + 1]["wg"][:, dk, :], GOUT[l + 1]["wg"][dk, :, :, :])
                        nc.sync.dma_start(W[l + 1]["wu"][:, dk, :], GOUT[l + 1]["wu"][dk, :, :, :])
            if t >= 1:
                emit_down(t - 1)
            for lw, k in ag_sched.get(t, ()):
                nc.gpsimd.collective_compute(
                    "AllGather",
                    ALU.bypass,
                    replica_groups=[list(range(n_cores))],
                    ins=[GIN[lw][k][:].opt()],
                    outs=[GOUT[lw][k][:].opt()],
                )
            if i == 0 and l >= 1:
                # wd refill for this layer; after down(l-1, 7) was emitted
                W[l]["wd"] = wts.tile([128, NFT, 1024], dt.float8e4, tag="wd", name="wd")
                if no_collectives:
                    for q_ in range(NCH):
                        nc.gpsimd.dma_start(W[l]["wd"][:, 8 * q_:8 * (q_ + 1), :], STG[l]["wd"][q_][:])
                else:
                    for c_ in range(n_cores):
                        nc.gpsimd.dma_start(W[l]["wd"][:, 4 * c_:4 * (c_ + 1), :], GOUT[l]["wd"][c_, :, :, :])
            if t == 0:
                # wsc2 broadcast here: the PE reaches it right as AllReduce #2
                # lands, without head-blocking the prologue matmuls
                wsc2_box[0] = emit_wmax_bcast("2", mx_glob2)
            # drain leftover background work
            while bgq:
                bgq.popleft()()
            while bgtail:
                bgtail.popleft()()
            if t == 0:
                _dbg("dbg_wd0", W[0]["wd"][:, 0:8, :], dt.float8e4, [128, 8, 1024])

        emit_down(nsteps - 1)

    if not no_collectives:  # sim-debug mode chokes on the injected waits
        _split_excess_waits(nc)
    return nc


_nc_cache = {}


def _get_nc(key=(NCORES, NTT, L)):
    if key not in _nc_cache:
        _nc_cache[key] = build(*key)
    return _nc_cache[key]


def _repack(w, n_layers):
    # [L, D, F] -> [L, F//1024, D//128, 128, 1024] so each (ch, dk) tile is
    # contiguous; dk == core index for the per-core max slices.
    return np.ascontiguousarray(
        w.reshape(n_layers, D // 128, 128, F // 1024, 1024).transpose(0, 3, 1, 2, 4)
    )


def _make_in_maps(x, rs, wg, wu, wd, n_cores=NCORES):
    n_layers = rs.shape[0]
    wg_r = _repack(wg, n_layers)
    wu_r = _repack(wu, n_layers)
    nsl = NDK // n_cores  # dk-slabs per core for slg/slu
    dsl = F // n_cores
    in_maps = []
    for c in range(n_cores):
        slg = wg_r[:, :, c * nsl:(c + 1) * nsl].reshape(n_layers, -1, 128, 1024)
        slu = wu_r[:, :, c * nsl:(c + 1) * nsl].reshape(n_layers, -1, 128, 1024)
        sld = wd[:, c * dsl:(c + 1) * dsl, :].reshape(n_layers, -1, 128, 1024)
        in_maps.append({
            "x": x[c],
            "rs": rs,
            "wg": wg_r,
            "wu": wu_r,
            "wd": wd,
            "slg": np.ascontiguousarray(slg),
            "slu": np.ascontiguousarray(slu),
            "sld": np.ascontiguousarray(sld),
        })
    return in_maps


def kernel(x, rms_scale, W_g, W_u, W_d):
    """Full-input entry point: shard over batch, run 8-core SPMD, gather."""
    x = np.ascontiguousarray(np.asarray(x, dtype=np.float32))
    rs = np.ascontiguousarray(np.asarray(rms_scale, dtype=np.float32))
    wg = np.ascontiguousarray(np.asarray(W_g, dtype=np.float32))
    wu = np.ascontiguousarray(np.asarray(W_u, dtype=np.float32))
    wd = np.ascontiguousarray(np.asarray(W_d, dtype=np.float32))
    B, S, Dx = x.shape
    assert (B, S, Dx) == (NCORES, TOK, D), (B, S, Dx)
    nc = _get_nc()
    in_maps = _make_in_maps(x, rs, wg, wu, wd)
    res = run_bass_kernel_spmd(nc, in_maps, list(range(NCORES)))
    return np.stack([res.results[c]["out"] for c in range(NCORES)], axis=0)
